# revision 4
# baseline (speedup 1.0000x reference)
"""Trainium2 Bass kernel for nn_CompressiveMemory_57750130262084.

The reference computes (B=8, S=4096, DK=DV=1024):
    sigma  = elu(query) + 1                                  [B,S,DK]
    memory = einsum('bkd,bsv->bkv', swap(sigma), value)      [B,DK,DV]
    z_norm = sum_s sigma                                     [B,DK]
    out    = einsum('bsd,bkv->bsv', sigma, memory)
           / einsum('bsd,bk->bs',  sigma, z_norm)[..., None]

Every einsum uses disjoint summed subscripts, so each factorises into
outer products of independent reductions:
    memory[b,k,v]    = z_norm[b,k] * VS[b,v]      with VS[b,v] = sum_s value[b,s,v]
    retrieved[b,s,v] = rs[b,s] * Z[b] * VS[b,v]   with rs = rowsum(sigma), Z = sum_k z_norm
    denom[b,s]       = rs[b,s] * Z[b]
    out[b,s,v]       = VS[b,v]                    (exactly; query cancels)

So the kernel is a column-sum of `value` over S, broadcast over S.
Sharding: data-parallel over batch, one NeuronCore per batch element.

Schedule per core (v3).  The 16 per-core DMA queues cap at ~26 GB/s
each (~417 GB/s aggregate, descriptor-size independent), so the floor
is read 16.8 MB + write fp16 8.4 MB back-to-back = ~60 us of DMA plus
prologue and the reduction tail.  Everything here minimises the tail:
  - p-major input layout: partition p holds 32 CONTIGUOUS DRAM rows
    [32p, 32p+32), so descriptors are rows-per-transfer x 4 KB.  Row
    placement is irrelevant: everything is summed.
  - input split across BOTH HWDGE engines (SP + Activation), rows
    0..15 / 16..31, transfer sizes [1,1,2,4,4,2,1,1] rows per engine:
    small head so the DVE add-chain starts at ~11 us, small tail so
    the last-arriving data is only 2 chunks.
  - per-[128,1024] chunk reduction split between DVE (fp32 tensor_add
    chain into acc) and PE (PSUM-accumulating ones^T @ chunk, which
    partition-reduces AND broadcasts), assignment hand-tuned so both
    engines idle just as the last pair lands; the last two chunks go
    c15->PE, c31->DVE, then ones^T @ acc folds the DVE accumulator
    into PSUM with per-bank stop flags.
  - output stored as float16 (tolerance 2e-2; fp16 adds ~1.6e-4 and
    the host upcasts), halving write traffic.  PSUM->SBUF conversion
    is four half-bank casts interleaved DVE/ACT behind the per-bank
    fold stops; they write TWO adjacent replicas so output descriptors
    are 4 KB.  ACT's activation-table load is pre-warmed at t~0.
  - output DMAs alternate between the two HWDGE engines.
"""

import numpy as np

B, S, D = 8, 4096, 1024
P = 128                 # SBUF partitions
RPP = S // P            # 32 rows per partition (p-major layout)
N_CHUNK = 32            # [128,1024] column chunks of the SBUF tile
GROUPS = [1, 1, 2, 4, 4, 2, 1, 1]  # rows/partition per transfer (per engine)
REP = 2                 # output row-replicas in SBUF -> 4KB descriptors
N_OUT = 16              # output transfers (256 rows each)
H = 512                 # PSUM bank width in f32 (matmul N limit)

_CACHE: dict = {}

# Arrival order: transfers complete pairwise (one per engine); sync
# carries chunks 0..15, scalar 16..31, groups [1,1,2,4,4,2,1,1].
ORDER = [
    0, 16, 1, 17,
    2, 3, 18, 19,
    4, 5, 6, 7, 20, 21, 22, 23,
    8, 9, 10, 11, 24, 25, 26, 27,
    12, 13, 28, 29,
    14, 30,
    15, 31,
]
# PE-owned chunks (greedy balance so both engines drain just after the
# last pair lands; DVE takes the final chunk c31, PE takes c15).
PE_CHUNKS = {16, 17, 3, 19, 6, 7, 21, 10, 11, 25, 12, 28, 30, 15}


def _build_program():
    import concourse.mybir as mybir
    import concourse.tile as tile
    from concourse import bacc

    f32 = mybir.dt.float32
    f16 = mybir.dt.float16
    assert sum(GROUPS) == 16
    nc = bacc.Bacc("TRN2", target_bir_lowering=False, debug=False, num_devices=B, enable_asserts=False)
    v = nc.declare_dram_parameter("value", [S, D], f32, isOutput=False)
    o = nc.declare_dram_parameter("out", [S, D], f16, isOutput=True)

    v_pm = v[:].rearrange("(p r) m -> p (r m)", p=P)       # [128][32*1024]
    o_re = o[:].rearrange("(i p n) m -> i p (n m)", p=P, n=REP)  # [16][128][2048]

    first_pe = next(c for c in ORDER if c in PE_CHUNKS)

    with tile.TileContext(nc) as tc:
        with (
            tc.tile_pool(name="in", bufs=1) as in_pool,
            tc.tile_pool(name="acc", bufs=1) as acc_pool,
            tc.tile_pool(name="ones", bufs=1) as ones_pool,
            tc.tile_pool(name="bcast", bufs=1) as bcast_pool,
            tc.tile_pool(name="warm", bufs=1) as warm_pool,
            tc.tile_pool(name="psum", bufs=1, space="PSUM") as psum_pool,
        ):
            ones = ones_pool.tile([P, P], f32)
            nc.vector.memset(ones[:], 1.0)
            warm = warm_pool.tile([P, 16], f32)
            nc.scalar.copy(warm[:], ones[:, 0:16])   # pre-warm ACT table load

            t = in_pool.tile([P, RPP * D], f32)
            ps = psum_pool.tile([P, D], f32)
            acc = acc_pool.tile([P, D], f32)

            # Input DMAs: each engine issues its transfers back-to-back.
            for half, eng in ((0, nc.sync), (1, nc.scalar)):
                r0 = half * 16
                for g in GROUPS:
                    sl = slice(r0 * D, (r0 + g) * D)
                    eng.dma_start(t[:, sl], v_pm[:, sl])
                    r0 += g

            # Reduction, issued in expected arrival order.
            n_dve = 0
            for c in ORDER:
                sl = t[:, c * D : (c + 1) * D]
                if c in PE_CHUNKS:
                    for h in range(2):
                        nc.tensor.matmul(
                            ps[:, h * H : (h + 1) * H],
                            ones[:],
                            sl[:, h * H : (h + 1) * H],
                            start=(c == first_pe),
                            stop=False,
                        )
                elif n_dve == 0:
                    nc.vector.tensor_copy(acc[:], sl)
                    n_dve += 1
                else:
                    nc.vector.tensor_add(acc[:], acc[:], sl)
                    n_dve += 1
            # Fold the DVE accumulator into PSUM; per-bank stop flags so
            # the casts below can start as soon as their bank settles.
            for h in range(2):
                nc.tensor.matmul(
                    ps[:, h * H : (h + 1) * H],
                    ones[:],
                    acc[:, h * H : (h + 1) * H],
                    start=False,
                    stop=True,
                )

            # PSUM -> SBUF f16, four half-bank casts interleaved DVE/ACT
            # writing two adjacent replicas (-> 4KB output descriptors).
            bc = bcast_pool.tile([P, REP * D], f16)
            nc.vector.tensor_copy(bc[:, 0:H], ps[:, 0:H])
            nc.scalar.copy(bc[:, D : D + H], ps[:, 0:H])
            nc.vector.tensor_copy(bc[:, D + H : 2 * D], ps[:, H:D])
            nc.scalar.copy(bc[:, H:D], ps[:, H:D])

            for i in range(N_OUT):
                eng = nc.sync if i % 2 == 0 else nc.scalar
                eng.dma_start(o_re[i], bc[:])

    nc.compile()
    return nc


def _get_program():
    if "nc" not in _CACHE:
        _CACHE["nc"] = _build_program()
    return _CACHE["nc"]


def kernel(query: np.ndarray, value: np.ndarray) -> np.ndarray:
    from concourse.bass_utils import run_bass_kernel_spmd

    del query  # output is exactly independent of query (see module docstring)
    value = np.ascontiguousarray(value, dtype=np.float32)
    assert value.shape == (B, S, D)

    nc = _get_program()
    in_maps = [{"value": value[b]} for b in range(B)]
    try:
        res = run_bass_kernel_spmd(nc, in_maps, list(range(B)))
    except Exception:
        # The tunneled runtime occasionally surfaces a transient
        # NRT_EXEC_UNIT_UNRECOVERABLE on the first dispatch; retry once.
        import time

        time.sleep(2.0)
        res = run_bass_kernel_spmd(nc, in_maps, list(range(B)))
    return np.stack(
        [res.results[b]["out"].astype(np.float32) for b in range(B)], axis=0
    )


# revision 5
# speedup vs baseline: 1.0363x; 1.0363x over previous
"""Trainium2 Bass kernel for nn_CompressiveMemory_57750130262084.

The reference computes (B=8, S=4096, DK=DV=1024):
    sigma  = elu(query) + 1                                  [B,S,DK]
    memory = einsum('bkd,bsv->bkv', swap(sigma), value)      [B,DK,DV]
    z_norm = sum_s sigma                                     [B,DK]
    out    = einsum('bsd,bkv->bsv', sigma, memory)
           / einsum('bsd,bk->bs',  sigma, z_norm)[..., None]

Every einsum uses disjoint summed subscripts, so each factorises into
outer products of independent reductions:
    memory[b,k,v]    = z_norm[b,k] * VS[b,v]      with VS[b,v] = sum_s value[b,s,v]
    retrieved[b,s,v] = rs[b,s] * Z[b] * VS[b,v]   with rs = rowsum(sigma), Z = sum_k z_norm
    denom[b,s]       = rs[b,s] * Z[b]
    out[b,s,v]       = VS[b,v]                    (exactly; query cancels)

So the kernel is a column-sum of `value` over S, broadcast over S.
Sharding: data-parallel over batch, one NeuronCore per batch element.

Schedule per core (v4).  The 16 per-core DMA queues cap at ~26 GB/s
each (~417 GB/s aggregate, descriptor-size independent), so the floor
is read 16.8 MB + write fp16 8.4 MB back-to-back = ~60 us of DMA plus
prologue and the reduction tail.  Everything here minimises the tail:
  - p-major input layout: partition p holds 32 CONTIGUOUS DRAM rows
    [32p, 32p+32), so descriptors are rows-per-transfer x 4 KB.  Row
    placement is irrelevant: everything is summed.
  - input split across BOTH HWDGE engines (SP + Activation), rows
    0..15 / 16..31, transfer sizes [1,1,2,4,4,2,1,1] rows per engine:
    small transfers at the head (compute starts ~11 us) and the tail
    (the final arrival is a single 1 MB pair).
  - chunk pair k = (sync row k, scalar row k+16) lands atomically; the
    DVE adds each pair into an independent tmp slot (NO serial
    accumulator chain -> no backlog behind a straggling transfer) and
    the PE accumulates ones^T @ tmp_k into PSUM, which partition-
    reduces AND broadcasts.  The last pair's matmul closes PSUM; there
    is no separate fold on the critical path.
  - output stored as float16 (tolerance 2e-2; fp16 adds ~1.6e-4 and
    the host upcasts), halving write traffic.  PSUM -> SBUF conversion
    is one DVE cast + one ACT copy (PSUM readers serialize anyway),
    writing TWO adjacent replicas so output descriptors are 4 KB.
    ACT's activation-table load is pre-warmed at t~0.
  - output DMAs alternate between the two HWDGE engines.
"""

import numpy as np

B, S, D = 8, 4096, 1024
P = 128                 # SBUF partitions
RPP = S // P            # 32 rows per partition (p-major layout)
N_PAIR = 16             # chunk pairs (one per row index 0..15)
GROUPS = [1, 1, 2, 4, 4, 2, 1, 1]  # rows/partition per transfer (per engine)
TMP_SLOTS = 4           # tmp ring depth (PE trails DVE by <2 pairs)
REP = 2                 # output row-replicas in SBUF -> 4KB descriptors
N_OUT = 16              # output transfers (256 rows each)
H = 512                 # PSUM bank width in f32 (matmul N limit)

_CACHE: dict = {}


def _build_program():
    import concourse.mybir as mybir
    import concourse.tile as tile
    from concourse import bacc

    f32 = mybir.dt.float32
    f16 = mybir.dt.float16
    assert sum(GROUPS) == 16
    nc = bacc.Bacc("TRN2", target_bir_lowering=False, debug=False, num_devices=B, enable_asserts=False)
    v = nc.declare_dram_parameter("value", [S, D], f32, isOutput=False)
    o = nc.declare_dram_parameter("out", [S, D], f16, isOutput=True)

    v_pm = v[:].rearrange("(p r) m -> p (r m)", p=P)       # [128][32*1024]
    o_re = o[:].rearrange("(i p n) m -> i p (n m)", p=P, n=REP)  # [16][128][2048]

    with tile.TileContext(nc) as tc:
        with (
            tc.tile_pool(name="in", bufs=1) as in_pool,
            tc.tile_pool(name="tmp", bufs=1) as tmp_pool,
            tc.tile_pool(name="ones", bufs=1) as ones_pool,
            tc.tile_pool(name="bcast", bufs=1) as bcast_pool,
            tc.tile_pool(name="warm", bufs=1) as warm_pool,
            tc.tile_pool(name="psum", bufs=1, space="PSUM") as psum_pool,
        ):
            ones = ones_pool.tile([P, P], f32)
            nc.vector.memset(ones[:], 1.0)
            warm = warm_pool.tile([P, 16], f32)
            nc.scalar.copy(warm[:], ones[:, 0:16])   # pre-warm ACT table load

            t = in_pool.tile([P, RPP * D], f32)
            tmp = tmp_pool.tile([P, TMP_SLOTS * D], f32)
            ps = psum_pool.tile([P, D], f32)

            # Input DMAs: each engine issues its transfers back-to-back.
            for half, eng in ((0, nc.sync), (1, nc.scalar)):
                r0 = half * 16
                for g in GROUPS:
                    sl = slice(r0 * D, (r0 + g) * D)
                    eng.dma_start(t[:, sl], v_pm[:, sl])
                    r0 += g

            # Reduction: pair k = rows (k, k+16) lands atomically.
            for k in range(N_PAIR):
                a = t[:, k * D : (k + 1) * D]
                b = t[:, (k + 16) * D : (k + 17) * D]
                s = k % TMP_SLOTS
                tk = tmp[:, s * D : (s + 1) * D]
                nc.vector.tensor_add(tk, a, b)
                for h in range(2):
                    nc.tensor.matmul(
                        ps[:, h * H : (h + 1) * H],
                        ones[:],
                        tk[:, h * H : (h + 1) * H],
                        start=(k == 0),
                        stop=(k == N_PAIR - 1),
                    )

            # PSUM -> SBUF f16, two replicas (-> 4KB output descriptors).
            bc = bcast_pool.tile([P, REP * D], f16)
            nc.vector.tensor_copy(bc[:, 0:D], ps[:])
            nc.scalar.copy(bc[:, D : 2 * D], ps[:])

            for i in range(N_OUT):
                eng = nc.sync if i % 2 == 0 else nc.scalar
                eng.dma_start(o_re[i], bc[:])

    nc.compile()
    return nc


def _get_program():
    if "nc" not in _CACHE:
        _CACHE["nc"] = _build_program()
    return _CACHE["nc"]


def kernel(query: np.ndarray, value: np.ndarray) -> np.ndarray:
    from concourse.bass_utils import run_bass_kernel_spmd

    del query  # output is exactly independent of query (see module docstring)
    value = np.ascontiguousarray(value, dtype=np.float32)
    assert value.shape == (B, S, D)

    nc = _get_program()
    in_maps = [{"value": value[b]} for b in range(B)]
    try:
        res = run_bass_kernel_spmd(nc, in_maps, list(range(B)))
    except Exception:
        # The tunneled runtime occasionally surfaces a transient
        # NRT_EXEC_UNIT_UNRECOVERABLE on the first dispatch; retry once.
        import time

        time.sleep(2.0)
        res = run_bass_kernel_spmd(nc, in_maps, list(range(B)))
    return np.stack(
        [res.results[b]["out"].astype(np.float32) for b in range(B)], axis=0
    )
